# revision 16
# baseline (speedup 1.0000x reference)
"""ConvLSTM3D encoder kernel for 8 trn2 NeuronCores (v2, bf16).

Sharding: core c in [0,8) handles batch b = c//4, z-slab k = c%4 (8 output
planes z in [8k, 8k+8)).  The sequential T=10 loop runs on-device; per-step
halo exchange (1 plane each side of the slab) is an AllGather over the 4
cores of each batch group (bf16 payload, double-buffered DRAM).

Conv mapping: gates = Wx (x) x_t (stride 2) + Wh (x) h + b is one K=128
bf16 matmul accumulation stream per output plane (N=1024 = full 32x32):
  partitions  0..95  : three z-shifted copies of h (dz = 0,1,2)
  partitions 96..122 : host-precomputed im2col taps of x_t (27 taps)
  partition  123     : ones (bias row, memset once)
For each (dy,dx) in 3x3, one matmul with an AP offset of (dy,dx) into the
padded (34x34) plane layout contracts channels x dz at once; the x-conv and
bias ride in the delta=(0,0) matmul only (their lhsT rows are zero in the
other eight).

Elementwise LSTM math runs on [32, span] slices straight out of the gates
tile (i/f/o/g live on partition quadrants 0:32/32:64/64:96/96:128 - the DVE
crossbar allows different quadrant bases per operand at nch=32), cell state
is fp16 for the 2-byte DVE fast mode.  h is written once (strided, bf16)
into the dz=1 partition group of the next h-stack; the dz=0/2 groups are
produced by two large contiguous SBUF->SBUF DMAs with a +-1 plane offset.
Boundary planes (0,7) are computed first each step so the halo collective
overlaps the interior-plane compute.
"""

import os
import sys
from contextlib import ExitStack

import numpy as np
import ml_dtypes

for _p in ("/opt/trn_rl_repo", "/root/.axon_site/_ro/trn_rl_repo"):
    if os.path.isdir(_p) and _p not in sys.path:
        sys.path.insert(0, _p)

import concourse.bass as bass
import concourse.bacc as bacc
import concourse.mybir as mybir
from concourse import tile
from concourse.bass_utils import run_bass_kernel_spmd

F32 = mybir.dt.float32
F16 = mybir.dt.float16
BF = mybir.dt.bfloat16
I32 = mybir.dt.int32

T = 10
CH = 32          # hidden channels
SLAB = 8         # output planes per core
PLW = 34         # padded plane width
PL = PLW * PLW   # padded plane elements (1156)
HS_FREE = SLAB * PL  # h-stack free size per partition (9248)
DELTAS = [(dy, dx) for dy in range(3) for dx in range(3)]
# plane processing order, chosen so each plane's stencil inputs (h[p-1],
# h[p], h[p+1]) are produced >=4 positions earlier in the previous step,
# and the halo planes (0,7) sit mid-order: late enough to consume the
# previous collective, early enough to feed this step's collective.
PO = [1, 2, 3, 0, 7, 4, 5, 6]
SPAN = {p: i * 1024 for i, p in enumerate(PO)}
RG = [[0, 1, 2, 3], [4, 5, 6, 7]]
NPBF = ml_dtypes.bfloat16

_prog_cache = {}


def _build_program(nsteps=T, halo=True, copies=True):
    key = (nsteps, halo, copies)
    if key in _prog_cache:
        return _prog_cache[key]

    nc = bacc.Bacc(num_devices=8)

    xim_d = nc.dram_tensor("xim", [T, 27, HS_FREE], BF, kind="ExternalInput")
    whl_d = nc.dram_tensor("whl", [9, 128, 128], BF, kind="ExternalInput")
    wxl_d = nc.dram_tensor("wxl", [28, 128], BF, kind="ExternalInput")
    hoff_d = nc.dram_tensor("hoff", [1, 2], I32, kind="ExternalInput")
    ones_d = nc.dram_tensor("ones", [1, HS_FREE], BF, kind="ExternalInput")
    hout_d = nc.dram_tensor("hout", [CH, SLAB, 32, 32], F32, kind="ExternalOutput")
    # A: everyone's h7 (lo-halos), B: everyone's h0 (hi-halos); slot 1 = zeros
    aginA = [nc.dram_tensor(f"aginA{i}", [2, CH, 1024], BF) for i in range(2)]
    agoutA = [nc.dram_tensor(f"agoutA{i}", [8, CH, 1024], BF) for i in range(2)]
    aginB = [nc.dram_tensor(f"aginB{i}", [2, CH, 1024], BF) for i in range(2)]
    agoutB = [nc.dram_tensor(f"agoutB{i}", [8, CH, 1024], BF) for i in range(2)]

    with ExitStack() as ctx:
        tc = ctx.enter_context(tile.TileContext(nc))
        pers = ctx.enter_context(tc.tile_pool(name="pers", bufs=1))
        psum = ctx.enter_context(tc.tile_pool(name="psum", bufs=4, space="PSUM"))
        work = ctx.enter_context(tc.tile_pool(name="work", bufs=1))

        hstack = [
            pers.tile([128, HS_FREE], BF, tag="hstackA", name="hstackA"),
            pers.tile([128, HS_FREE], BF, tag="hstackB", name="hstackB"),
        ]
        xim_sb = [
            pers.tile([28, HS_FREE], BF, tag="ximA", name="ximA"),
            pers.tile([28, HS_FREE], BF, tag="ximB", name="ximB"),
        ]
        wh_sb = pers.tile([128, 9 * 128], BF, tag="wh")
        wx_sb = pers.tile([28, 128], BF, tag="wx")
        # elementwise operand placement: every two-input DVE op needs both
        # inputs on the same base partition, so: g at base 0 (own tile),
        # c/prod/tmp at base 32, tanh(c) at base 64 (next to o).
        c_state = pers.tile([64, 8 * 1024], F16, tag="cstate")
        prod = pers.tile([64, 8 * 1024], F16, tag="prod")
        tmp = pers.tile([64, 8 * 1024], F16, tag="tmp")
        tanhc = pers.tile([96, 8 * 1024], F16, tag="tanhc")
        hfin = pers.tile([32, 8 * 1024], F32, tag="hfin")
        zscr = pers.tile([CH, 1024], BF, tag="zscr")

        # ---- init ----
        nc.vector.memset(hstack[0][:, :], 0.0)
        nc.gpsimd.memset(hstack[1][:, :], 0.0)
        nc.vector.memset(c_state[32:64, :], 0.0)
        nc.vector.memset(zscr[:, :], 0.0)
        for i in range(2):
            nc.sync.dma_start(out=xim_sb[i][27:28, :], in_=ones_d[:, :])
        nc.sync.dma_start(out=wx_sb[:, :], in_=wxl_d[:, :])
        for i in range(2):
            nc.sync.dma_start(out=aginA[i][1], in_=zscr[:, :])
            nc.sync.dma_start(out=aginB[i][1], in_=zscr[:, :])
        for _d in range(9):
            nc.sync.dma_start(out=wh_sb[:, 128 * _d:128 * (_d + 1)],
                              in_=whl_d[_d])
        nc.sync.dma_start(out=xim_sb[0][0:27, :], in_=xim_d[0])

        r_lo = nc.alloc_register(mybir.EngineType.Pool, "r_lo")
        r_hi = nc.alloc_register(mybir.EngineType.Pool, "r_hi")
        nc.reg_load(r_lo, hoff_d[0:1, 0:1])
        nc.reg_load(r_hi, hoff_d[0:1, 1:2])
        rv_lo = nc.snap(r_lo, min_val=0, max_val=7)
        rv_hi = nc.snap(r_hi, min_val=0, max_val=7)

        hsv = [h[:, :].rearrange("p (z y x) -> p z y x", z=SLAB, y=PLW, x=PLW)
               for h in hstack]
        xsv = [x[:, :].rearrange("p (z y x) -> p z y x", z=SLAB, y=PLW, x=PLW)
               for x in xim_sb]

        def emit_plane(t, curv, curx, gates, p):
            """x-conv + 9-delta h-conv matmul accumulation + activations."""
            ps = psum.tile([128, 1024], F32, tag="ps", name="ps")
            for h in range(2):
                nc.tensor.matmul(ps[:, 512 * h:512 * (h + 1)],
                                 lhsT=wx_sb[:, :],
                                 rhs=curx[:, p, 16 * h:16 * h + 16, 0:32],
                                 start=True, stop=(t == 0))
            if t > 0:
                for di, (dy, dx) in enumerate(DELTAS):
                    for h in range(2):
                        nc.tensor.matmul(
                            ps[:, 512 * h:512 * (h + 1)],
                            lhsT=wh_sb[:, 128 * di:128 * (di + 1)],
                            rhs=curv[:, p, 16 * h + dy:16 * h + dy + 16,
                                     dx:dx + 32],
                            start=False, stop=(di == 8))
            s = SPAN[p]
            nc.scalar.activation(gates[0:96, s:s + 1024], ps[0:96, :],
                                 mybir.ActivationFunctionType.Sigmoid)
            nc.scalar.activation(g_t[0:32, s:s + 1024], ps[96:128, :],
                                 mybir.ActivationFunctionType.Tanh)

        def emit_group(t, gates, g_t, nxtv, planes, s0, s1):
            """LSTM elementwise update for gate span [s0:s1] (planes list)."""
            i_sl = gates[0:32, s0:s1]
            f_sl = gates[32:64, s0:s1]
            o_sl = gates[64:96, s0:s1]
            c_sl = c_state[32:64, s0:s1]
            nc.vector.tensor_mul(prod[32:64, s0:s1], i_sl, g_t[0:32, s0:s1])
            nc.vector.tensor_mul(tmp[32:64, s0:s1], f_sl, c_sl)
            nc.vector.tensor_add(c_sl, tmp[32:64, s0:s1], prod[32:64, s0:s1])
            nc.scalar.activation(tanhc[64:96, s0:s1], c_sl,
                                 mybir.ActivationFunctionType.Tanh)
            last = t == nsteps - 1
            for pl, a, b in planes:
                o_ap = o_sl[:, a - s0:b - s0].rearrange(
                    "p (z y x) -> p z y x", z=(b - a) // 1024, y=32, x=32)
                t_ap = tanhc[64:96, a:b].rearrange(
                    "p (z y x) -> p z y x", z=(b - a) // 1024, y=32, x=32)
                if last:
                    nc.vector.tensor_mul(
                        hfin[:, a:b].rearrange("p (z y x) -> p z y x",
                                               z=(b - a) // 1024, y=32, x=32),
                        o_ap, t_ap)
                else:
                    npl = (b - a) // 1024
                    nc.vector.tensor_mul(
                        nxtv[32:64, pl:pl + npl, 1:33, 1:33], o_ap, t_ap)

        T_ = nsteps
        for t in range(T_):
            curv, nxtv = hsv[t % 2], hsv[(t + 1) % 2]
            curx = xsv[t % 2]
            last = t == T_ - 1
            gates = work.tile([96, 8 * 1024], BF, tag="gates", name="gates")
            g_t = work.tile([32, 8 * 1024], BF, tag="g_t", name="g_t")
            if not last:
                nc.sync.dma_start(out=xim_sb[(t + 1) % 2][0:27, :],
                                  in_=xim_d[t + 1])

            # process planes in pair-chunks in PO order; after each chunk,
            # run its elementwise update and issue the dz +-1 replica copies
            # for every slot whose source plane is now available
            agiA, agoA = aginA[t % 2], agoutA[t % 2]
            agiB, agoB = aginB[t % 2], agoutB[t % 2]
            # g0 slot p+1 <- h[p] copy spans, g2 slot p-1 <- h[p], per chunk
            CPY = {
                1: [(0, 2, 4, 1, 3), (2, 0, 2, 1, 3)],
                3: [(0, 4, 5, 3, 4), (2, 2, 3, 3, 4), (0, 1, 2, 0, 1)],
                7: [(2, 6, 7, 7, 8), (0, 5, 6, 4, 5), (2, 3, 4, 4, 5)],
                5: [(0, 6, 8, 5, 7), (2, 4, 6, 5, 7)],
            }
            for ci in range(4):
                p0, p1 = PO[2 * ci], PO[2 * ci + 1]
                emit_plane(t, curv, curx, gates, p0)
                emit_plane(t, curv, curx, gates, p1)
                s0 = SPAN[p0]
                planes = ([(p0, s0, s0 + 2048)] if p1 == p0 + 1 else
                          [(p0, s0, s0 + 1024), (p1, s0 + 1024, s0 + 2048)])
                emit_group(t, gates, g_t, nxtv, planes, s0, s0 + 2048)
                if last:
                    continue
                for grp, d0, d1, src0, src1 in (CPY[p0] if copies else []):
                    gp = 0 if grp == 0 else 64
                    nc.scalar.dma_start(out=nxtv[gp:gp + 32, d0:d1, :, :],
                                        in_=nxtv[32:64, src0:src1, :, :])
                if p0 == 3:  # h0 just produced -> gather for hi-halos
                    nc.scalar.dma_start(
                        out=agiB[0].rearrange("c (y x) -> c y x", y=32, x=32),
                        in_=nxtv[32:64, 0, 1:33, 1:33])
                    if halo:
                        nc.gpsimd.collective_compute(
                            "AllGather", mybir.AluOpType.bypass,
                            replica_groups=RG,
                            ins=[agiB[:, :, :]], outs=[agoB[:, :, :]])
                elif p0 == 7:  # h7 just produced -> gather for lo-halos
                    nc.scalar.dma_start(
                        out=agiA[0].rearrange("c (y x) -> c y x", y=32, x=32),
                        in_=nxtv[32:64, 7, 1:33, 1:33])
                    if halo:
                        nc.gpsimd.collective_compute(
                            "AllGather", mybir.AluOpType.bypass,
                            replica_groups=RG,
                            ins=[agiA[:, :, :]], outs=[agoA[:, :, :]])

            if not last and halo:
                halo_lo = agoA[bass.ds(rv_lo, 1)].squeeze(0).rearrange(
                    "c (y x) -> c y x", y=32, x=32)
                halo_hi = agoB[bass.ds(rv_hi, 1)].squeeze(0).rearrange(
                    "c (y x) -> c y x", y=32, x=32)
                nc.gpsimd.dma_start(out=nxtv[0:32, 0, 1:33, 1:33], in_=halo_lo)
                nc.gpsimd.dma_start(out=nxtv[64:96, 7, 1:33, 1:33], in_=halo_hi)
            else:
                for pl in range(SLAB):
                    s = SPAN[pl]
                    nc.sync.dma_start(
                        out=hout_d[:, pl, :, :],
                        in_=hfin[:, s:s + 1024].rearrange(
                            "c (y x) -> c y x", y=32, x=32))

    nc.finalize()
    _prog_cache[key] = nc
    return nc


def _host_inputs(input_batch, Wx, Wh, b):
    input_batch = np.asarray(input_batch, dtype=np.float32)
    Wx = np.asarray(Wx, dtype=np.float32)
    Wh = np.asarray(Wh, dtype=np.float32)
    b = np.asarray(b, dtype=np.float32)

    xp = np.zeros((2, T, 66, 66, 66), np.float32)
    xp[:, :, 1:65, 1:65, 1:65] = input_batch[:, :, 0]

    whl = np.zeros((9, 128, 128), np.float32)
    for di, (dy, dx) in enumerate(DELTAS):
        for g in range(3):
            whl[di, 32 * g:32 * g + 32, :] = Wh[:, :, g, dy, dx].T
    whl = whl.astype(NPBF)
    wxl = np.zeros((28, 128), np.float32)
    wxl[0:27, :] = Wx[:, 0].reshape(128, 27).T
    wxl[27, :] = b
    wxl = wxl.astype(NPBF)

    in_maps = []
    for c in range(8):
        bidx, k = divmod(c, 4)
        z0 = 8 * k
        xim = np.zeros((T, 27, SLAB, PLW, PLW), np.float32)
        for tz in range(3):
            for ty in range(3):
                for tx in range(3):
                    tap = tz * 9 + ty * 3 + tx
                    xim[:, tap, :, 0:32, 0:32] = xp[
                        bidx, :, 2 * z0 + tz:2 * z0 + tz + 16:2,
                        ty:ty + 64:2, tx:tx + 64:2]
        lo_slot = 1 if k == 0 else 2 * (k - 1)
        hi_slot = 1 if k == 3 else 2 * (k + 1)
        in_maps.append({
            "xim": xim.reshape(T, 27, HS_FREE).astype(NPBF),
            "whl": whl,
            "wxl": wxl,
            "ones": np.ones((1, HS_FREE), NPBF),
            "hoff": np.array([[lo_slot, hi_slot]], np.int32),
        })
    return in_maps


def run_cores(in_maps, nsteps=T, halo=True, copies=True, **kwargs):
    nc = _build_program(nsteps, halo, copies)
    return run_bass_kernel_spmd(nc, in_maps, list(range(8)), **kwargs)


def kernel(input_batch, Wx, Wh, b):
    in_maps = _host_inputs(input_batch, Wx, Wh, b)
    res = run_cores(in_maps)
    out = np.zeros((2, CH, 32, 32, 32), np.float32)
    for c in range(8):
        bidx, k = divmod(c, 4)
        out[bidx, :, 8 * k:8 * k + 8] = res.results[c]["hout"]
    return out


# revision 17
# speedup vs baseline: 1.0858x; 1.0858x over previous
"""ConvLSTM3D encoder kernel for 8 trn2 NeuronCores (v2, bf16).

Sharding: core c in [0,8) handles batch b = c//4, z-slab k = c%4 (8 output
planes z in [8k, 8k+8)).  The sequential T=10 loop runs on-device; per-step
halo exchange (1 plane each side of the slab) is an AllGather over the 4
cores of each batch group (bf16 payload, double-buffered DRAM).

Conv mapping: gates = Wx (x) x_t (stride 2) + Wh (x) h + b is one K=128
bf16 matmul accumulation stream per output plane (N=1024 = full 32x32):
  partitions  0..95  : three z-shifted copies of h (dz = 0,1,2)
  partitions 96..122 : host-precomputed im2col taps of x_t (27 taps)
  partition  123     : ones (bias row, memset once)
For each (dy,dx) in 3x3, one matmul with an AP offset of (dy,dx) into the
padded (34x34) plane layout contracts channels x dz at once; the x-conv and
bias ride in the delta=(0,0) matmul only (their lhsT rows are zero in the
other eight).

Elementwise LSTM math runs on [32, span] slices straight out of the gates
tile (i/f/o/g live on partition quadrants 0:32/32:64/64:96/96:128 - the DVE
crossbar allows different quadrant bases per operand at nch=32), cell state
is fp16 for the 2-byte DVE fast mode.  h is written once (strided, bf16)
into the dz=1 partition group of the next h-stack; the dz=0/2 groups are
produced by two large contiguous SBUF->SBUF DMAs with a +-1 plane offset.
Boundary planes (0,7) are computed first each step so the halo collective
overlaps the interior-plane compute.
"""

import os
import sys
from contextlib import ExitStack

import numpy as np
import ml_dtypes

for _p in ("/opt/trn_rl_repo", "/root/.axon_site/_ro/trn_rl_repo"):
    if os.path.isdir(_p) and _p not in sys.path:
        sys.path.insert(0, _p)

import concourse.bass as bass
import concourse.bacc as bacc
import concourse.mybir as mybir
from concourse import tile
from concourse.bass_utils import run_bass_kernel_spmd

F32 = mybir.dt.float32
F16 = mybir.dt.float16
BF = mybir.dt.bfloat16
I32 = mybir.dt.int32

T = 10
CH = 32          # hidden channels
SLAB = 8         # output planes per core
PLW = 34         # padded plane width
PL = PLW * PLW   # padded plane elements (1156)
HS_FREE = SLAB * PL  # h-stack free size per partition (9248)
DELTAS = [(dy, dx) for dy in range(3) for dx in range(3)]
# plane processing order, chosen so each plane's stencil inputs (h[p-1],
# h[p], h[p+1]) are produced >=4 positions earlier in the previous step,
# and the halo planes (0,7) sit mid-order: late enough to consume the
# previous collective, early enough to feed this step's collective.
PO = [1, 2, 3, 4, 5, 6, 0, 7]
SPAN = {p: i * 1024 for i, p in enumerate(PO)}
RG = [[0, 1, 2, 3], [4, 5, 6, 7]]
NPBF = ml_dtypes.bfloat16

_prog_cache = {}


def _build_program(nsteps=T, halo=True, copies=True):
    key = (nsteps, halo, copies)
    if key in _prog_cache:
        return _prog_cache[key]

    nc = bacc.Bacc(num_devices=8)

    xim_d = nc.dram_tensor("xim", [T, 27, HS_FREE], BF, kind="ExternalInput")
    whl_d = nc.dram_tensor("whl", [9, 128, 128], BF, kind="ExternalInput")
    wxl_d = nc.dram_tensor("wxl", [28, 128], BF, kind="ExternalInput")
    hoff_d = nc.dram_tensor("hoff", [1, 2], I32, kind="ExternalInput")
    ones_d = nc.dram_tensor("ones", [1, HS_FREE], BF, kind="ExternalInput")
    hout_d = nc.dram_tensor("hout", [CH, SLAB, 32, 32], F32, kind="ExternalOutput")
    agin = [nc.dram_tensor(f"agin{i}", [3, CH, 1024], BF) for i in range(2)]
    agout = [nc.dram_tensor(f"agout{i}", [12, CH, 1024], BF) for i in range(2)]

    with ExitStack() as ctx:
        tc = ctx.enter_context(tile.TileContext(nc))
        pers = ctx.enter_context(tc.tile_pool(name="pers", bufs=1))
        psum = ctx.enter_context(tc.tile_pool(name="psum", bufs=4, space="PSUM"))
        work = ctx.enter_context(tc.tile_pool(name="work", bufs=1))

        hstack = [
            pers.tile([128, HS_FREE], BF, tag="hstackA", name="hstackA"),
            pers.tile([128, HS_FREE], BF, tag="hstackB", name="hstackB"),
        ]
        xim_sb = [
            pers.tile([28, HS_FREE], BF, tag="ximA", name="ximA"),
            pers.tile([28, HS_FREE], BF, tag="ximB", name="ximB"),
        ]
        wh_sb = pers.tile([128, 9 * 128], BF, tag="wh")
        wx_sb = pers.tile([28, 128], BF, tag="wx")
        # elementwise operand placement: every two-input DVE op needs both
        # inputs on the same base partition, so: g at base 0 (own tile),
        # c/prod/tmp at base 32, tanh(c) at base 64 (next to o).
        c_state = pers.tile([64, 8 * 1024], F16, tag="cstate")
        prod = pers.tile([64, 8 * 1024], F16, tag="prod")
        tmp = pers.tile([64, 8 * 1024], F16, tag="tmp")
        tanhc = pers.tile([96, 8 * 1024], F16, tag="tanhc")
        hfin = pers.tile([32, 8 * 1024], F32, tag="hfin")
        zscr = pers.tile([CH, 1024], BF, tag="zscr")

        # ---- init ----
        nc.vector.memset(hstack[0][:, :], 0.0)
        nc.gpsimd.memset(hstack[1][:, :], 0.0)
        nc.vector.memset(c_state[32:64, :], 0.0)
        nc.vector.memset(zscr[:, :], 0.0)
        for i in range(2):
            nc.sync.dma_start(out=xim_sb[i][27:28, :], in_=ones_d[:, :])
        nc.sync.dma_start(out=wx_sb[:, :], in_=wxl_d[:, :])
        for i in range(2):
            nc.sync.dma_start(out=agin[i][2], in_=zscr[:, :])
        for _d in range(9):
            nc.sync.dma_start(out=wh_sb[:, 128 * _d:128 * (_d + 1)],
                              in_=whl_d[_d])
        nc.sync.dma_start(out=xim_sb[0][0:27, :], in_=xim_d[0])

        r_lo = nc.alloc_register(mybir.EngineType.Pool, "r_lo")
        r_hi = nc.alloc_register(mybir.EngineType.Pool, "r_hi")
        nc.reg_load(r_lo, hoff_d[0:1, 0:1])
        nc.reg_load(r_hi, hoff_d[0:1, 1:2])
        rv_lo = nc.snap(r_lo, min_val=0, max_val=11)
        rv_hi = nc.snap(r_hi, min_val=0, max_val=11)

        hsv = [h[:, :].rearrange("p (z y x) -> p z y x", z=SLAB, y=PLW, x=PLW)
               for h in hstack]
        xsv = [x[:, :].rearrange("p (z y x) -> p z y x", z=SLAB, y=PLW, x=PLW)
               for x in xim_sb]

        def emit_plane(t, curv, curx, gates, p):
            """x-conv + 9-delta h-conv matmul accumulation + activations."""
            ps = psum.tile([128, 1024], F32, tag="ps", name="ps")
            for h in range(2):
                nc.tensor.matmul(ps[:, 512 * h:512 * (h + 1)],
                                 lhsT=wx_sb[:, :],
                                 rhs=curx[:, p, 16 * h:16 * h + 16, 0:32],
                                 start=True, stop=(t == 0))
            if t > 0:
                for di, (dy, dx) in enumerate(DELTAS):
                    for h in range(2):
                        nc.tensor.matmul(
                            ps[:, 512 * h:512 * (h + 1)],
                            lhsT=wh_sb[:, 128 * di:128 * (di + 1)],
                            rhs=curv[:, p, 16 * h + dy:16 * h + dy + 16,
                                     dx:dx + 32],
                            start=False, stop=(di == 8))
            s = SPAN[p]
            nc.scalar.activation(gates[0:96, s:s + 1024], ps[0:96, :],
                                 mybir.ActivationFunctionType.Sigmoid)
            nc.scalar.activation(g_t[0:32, s:s + 1024], ps[96:128, :],
                                 mybir.ActivationFunctionType.Tanh)

        def emit_group(t, gates, g_t, nxtv, planes, s0, s1):
            """LSTM elementwise update for gate span [s0:s1] (planes list)."""
            i_sl = gates[0:32, s0:s1]
            f_sl = gates[32:64, s0:s1]
            o_sl = gates[64:96, s0:s1]
            c_sl = c_state[32:64, s0:s1]
            nc.vector.tensor_mul(prod[32:64, s0:s1], i_sl, g_t[0:32, s0:s1])
            nc.vector.tensor_mul(tmp[32:64, s0:s1], f_sl, c_sl)
            nc.vector.tensor_add(c_sl, tmp[32:64, s0:s1], prod[32:64, s0:s1])
            nc.scalar.activation(tanhc[64:96, s0:s1], c_sl,
                                 mybir.ActivationFunctionType.Tanh)
            last = t == nsteps - 1
            for pl, a, b in planes:
                o_ap = o_sl[:, a - s0:b - s0].rearrange(
                    "p (z y x) -> p z y x", z=(b - a) // 1024, y=32, x=32)
                t_ap = tanhc[64:96, a:b].rearrange(
                    "p (z y x) -> p z y x", z=(b - a) // 1024, y=32, x=32)
                if last:
                    nc.vector.tensor_mul(
                        hfin[:, a:b].rearrange("p (z y x) -> p z y x",
                                               z=(b - a) // 1024, y=32, x=32),
                        o_ap, t_ap)
                else:
                    npl = (b - a) // 1024
                    nc.vector.tensor_mul(
                        nxtv[32:64, pl:pl + npl, 1:33, 1:33], o_ap, t_ap)

        T_ = nsteps
        for t in range(T_):
            curv, nxtv = hsv[t % 2], hsv[(t + 1) % 2]
            curx = xsv[t % 2]
            last = t == T_ - 1
            gates = work.tile([96, 8 * 1024], BF, tag="gates", name="gates")
            g_t = work.tile([32, 8 * 1024], BF, tag="g_t", name="g_t")
            if not last:
                nc.sync.dma_start(out=xim_sb[(t + 1) % 2][0:27, :],
                                  in_=xim_d[t + 1])

            # process planes in pair-chunks in PO order; after each chunk,
            # run its elementwise update and issue the dz +-1 replica copies
            # for every slot whose source plane is now available
            ag_i, ag_o = agin[t % 2], agout[t % 2]
            # g0 slot p+1 <- h[p] copy spans, g2 slot p-1 <- h[p], per chunk
            CPY = {
                1: [(0, 2, 4, 1, 3), (2, 0, 2, 1, 3)],
                3: [(0, 4, 6, 3, 5), (2, 2, 4, 3, 5)],
                5: [(0, 6, 8, 5, 7), (2, 4, 6, 5, 7)],
                0: [(0, 1, 2, 0, 1), (2, 6, 7, 7, 8)],
            }
            for ci in range(4):
                p0, p1 = PO[2 * ci], PO[2 * ci + 1]
                emit_plane(t, curv, curx, gates, p0)
                emit_plane(t, curv, curx, gates, p1)
                s0 = SPAN[p0]
                planes = ([(p0, s0, s0 + 2048)] if p1 == p0 + 1 else
                          [(p0, s0, s0 + 1024), (p1, s0 + 1024, s0 + 2048)])
                emit_group(t, gates, g_t, nxtv, planes, s0, s0 + 2048)
                if last:
                    continue
                for grp, d0, d1, src0, src1 in (CPY[p0] if copies else []):
                    gp = 0 if grp == 0 else 64
                    nc.scalar.dma_start(out=nxtv[gp:gp + 32, d0:d1, :, :],
                                        in_=nxtv[32:64, src0:src1, :, :])
                if p0 == 0:  # boundary chunk is last: h0 and h7 both ready
                    nc.scalar.dma_start(
                        out=ag_i[0].rearrange("c (y x) -> c y x", y=32, x=32),
                        in_=nxtv[32:64, 0, 1:33, 1:33])
                    nc.scalar.dma_start(
                        out=ag_i[1].rearrange("c (y x) -> c y x", y=32, x=32),
                        in_=nxtv[32:64, 7, 1:33, 1:33])
                    if halo:
                        nc.gpsimd.collective_compute(
                            "AllGather", mybir.AluOpType.bypass,
                            replica_groups=RG,
                            ins=[ag_i[:, :, :]], outs=[ag_o[:, :, :]])

            if not last and halo:
                halo_lo = ag_o[bass.ds(rv_lo, 1)].squeeze(0).rearrange(
                    "c (y x) -> c y x", y=32, x=32)
                halo_hi = ag_o[bass.ds(rv_hi, 1)].squeeze(0).rearrange(
                    "c (y x) -> c y x", y=32, x=32)
                nc.gpsimd.dma_start(out=nxtv[0:32, 0, 1:33, 1:33], in_=halo_lo)
                nc.gpsimd.dma_start(out=nxtv[64:96, 7, 1:33, 1:33], in_=halo_hi)
            else:
                for pl in range(SLAB):
                    s = SPAN[pl]
                    nc.sync.dma_start(
                        out=hout_d[:, pl, :, :],
                        in_=hfin[:, s:s + 1024].rearrange(
                            "c (y x) -> c y x", y=32, x=32))

    nc.finalize()
    _prog_cache[key] = nc
    return nc


def _host_inputs(input_batch, Wx, Wh, b):
    input_batch = np.asarray(input_batch, dtype=np.float32)
    Wx = np.asarray(Wx, dtype=np.float32)
    Wh = np.asarray(Wh, dtype=np.float32)
    b = np.asarray(b, dtype=np.float32)

    xp = np.zeros((2, T, 66, 66, 66), np.float32)
    xp[:, :, 1:65, 1:65, 1:65] = input_batch[:, :, 0]

    whl = np.zeros((9, 128, 128), np.float32)
    for di, (dy, dx) in enumerate(DELTAS):
        for g in range(3):
            whl[di, 32 * g:32 * g + 32, :] = Wh[:, :, g, dy, dx].T
    whl = whl.astype(NPBF)
    wxl = np.zeros((28, 128), np.float32)
    wxl[0:27, :] = Wx[:, 0].reshape(128, 27).T
    wxl[27, :] = b
    wxl = wxl.astype(NPBF)

    in_maps = []
    for c in range(8):
        bidx, k = divmod(c, 4)
        z0 = 8 * k
        xim = np.zeros((T, 27, SLAB, PLW, PLW), np.float32)
        for tz in range(3):
            for ty in range(3):
                for tx in range(3):
                    tap = tz * 9 + ty * 3 + tx
                    xim[:, tap, :, 0:32, 0:32] = xp[
                        bidx, :, 2 * z0 + tz:2 * z0 + tz + 16:2,
                        ty:ty + 64:2, tx:tx + 64:2]
        lo_slot = 3 * k + 2 if k == 0 else 3 * (k - 1) + 1
        hi_slot = 3 * k + 2 if k == 3 else 3 * (k + 1)
        in_maps.append({
            "xim": xim.reshape(T, 27, HS_FREE).astype(NPBF),
            "whl": whl,
            "wxl": wxl,
            "ones": np.ones((1, HS_FREE), NPBF),
            "hoff": np.array([[lo_slot, hi_slot]], np.int32),
        })
    return in_maps


def run_cores(in_maps, nsteps=T, halo=True, copies=True, **kwargs):
    nc = _build_program(nsteps, halo, copies)
    return run_bass_kernel_spmd(nc, in_maps, list(range(8)), **kwargs)


def kernel(input_batch, Wx, Wh, b):
    in_maps = _host_inputs(input_batch, Wx, Wh, b)
    res = run_cores(in_maps)
    out = np.zeros((2, CH, 32, 32, 32), np.float32)
    for c in range(8):
        bidx, k = divmod(c, 4)
        out[bidx, :, 8 * k:8 * k + 8] = res.results[c]["hout"]
    return out


# revision 18
# speedup vs baseline: 1.1068x; 1.0194x over previous
"""ConvLSTM3D encoder kernel for 8 trn2 NeuronCores (v2, bf16).

Sharding: core c in [0,8) handles batch b = c//4, z-slab k = c%4 (8 output
planes z in [8k, 8k+8)).  The sequential T=10 loop runs on-device; per-step
halo exchange (1 plane each side of the slab) is an AllGather over the 4
cores of each batch group (bf16 payload, double-buffered DRAM).

Conv mapping: gates = Wx (x) x_t (stride 2) + Wh (x) h + b is one K=128
bf16 matmul accumulation stream per output plane (N=1024 = full 32x32):
  partitions  0..95  : three z-shifted copies of h (dz = 0,1,2)
  partitions 96..122 : host-precomputed im2col taps of x_t (27 taps)
  partition  123     : ones (bias row, memset once)
For each (dy,dx) in 3x3, one matmul with an AP offset of (dy,dx) into the
padded (34x34) plane layout contracts channels x dz at once; the x-conv and
bias ride in the delta=(0,0) matmul only (their lhsT rows are zero in the
other eight).

Elementwise LSTM math runs on [32, span] slices straight out of the gates
tile (i/f/o/g live on partition quadrants 0:32/32:64/64:96/96:128 - the DVE
crossbar allows different quadrant bases per operand at nch=32), cell state
is fp16 for the 2-byte DVE fast mode.  h is written once (strided, bf16)
into the dz=1 partition group of the next h-stack; the dz=0/2 groups are
produced by two large contiguous SBUF->SBUF DMAs with a +-1 plane offset.
Boundary planes (0,7) are computed first each step so the halo collective
overlaps the interior-plane compute.
"""

import os
import sys
from contextlib import ExitStack

import numpy as np
import ml_dtypes

for _p in ("/opt/trn_rl_repo", "/root/.axon_site/_ro/trn_rl_repo"):
    if os.path.isdir(_p) and _p not in sys.path:
        sys.path.insert(0, _p)

import concourse.bass as bass
import concourse.bacc as bacc
import concourse.mybir as mybir
from concourse import tile
from concourse.bass_utils import run_bass_kernel_spmd

F32 = mybir.dt.float32
F16 = mybir.dt.float16
BF = mybir.dt.bfloat16
I32 = mybir.dt.int32

T = 10
CH = 32          # hidden channels
SLAB = 8         # output planes per core
PLW = 34         # padded plane width
PL = PLW * PLW   # padded plane elements (1156)
HS_FREE = SLAB * PL  # h-stack free size per partition (9248)
DELTAS = [(dy, dx) for dy in range(3) for dx in range(3)]
# plane processing order, chosen so each plane's stencil inputs (h[p-1],
# h[p], h[p+1]) are produced >=4 positions earlier in the previous step,
# and the halo planes (0,7) sit mid-order: late enough to consume the
# previous collective, early enough to feed this step's collective.
PO = [1, 2, 3, 4, 5, 6, 0, 7]  # span layout; processing order differs
SPAN = {p: i * 1024 for i, p in enumerate(PO)}
RG = [[0, 1, 2, 3], [4, 5, 6, 7]]
NPBF = ml_dtypes.bfloat16

_prog_cache = {}


def _build_program(nsteps=T, halo=True, copies=True):
    key = (nsteps, halo, copies)
    if key in _prog_cache:
        return _prog_cache[key]

    nc = bacc.Bacc(num_devices=8)

    xim_d = nc.dram_tensor("xim", [T, 27, HS_FREE], BF, kind="ExternalInput")
    whl_d = nc.dram_tensor("whl", [9, 128, 128], BF, kind="ExternalInput")
    wxl_d = nc.dram_tensor("wxl", [28, 128], BF, kind="ExternalInput")
    hoff_d = nc.dram_tensor("hoff", [1, 2], I32, kind="ExternalInput")
    ones_d = nc.dram_tensor("ones", [1, HS_FREE], BF, kind="ExternalInput")
    hout_d = nc.dram_tensor("hout", [CH, SLAB, 32, 32], F32, kind="ExternalOutput")
    agin = [nc.dram_tensor(f"agin{i}", [3, CH, 1024], BF) for i in range(2)]
    agout = [nc.dram_tensor(f"agout{i}", [12, CH, 1024], BF) for i in range(2)]

    with ExitStack() as ctx:
        tc = ctx.enter_context(tile.TileContext(nc))
        pers = ctx.enter_context(tc.tile_pool(name="pers", bufs=1))
        psum = ctx.enter_context(tc.tile_pool(name="psum", bufs=4, space="PSUM"))
        work = ctx.enter_context(tc.tile_pool(name="work", bufs=1))

        hstack = [
            pers.tile([128, HS_FREE], BF, tag="hstackA", name="hstackA"),
            pers.tile([128, HS_FREE], BF, tag="hstackB", name="hstackB"),
        ]
        xim_sb = [
            pers.tile([28, HS_FREE], BF, tag="ximA", name="ximA"),
            pers.tile([28, HS_FREE], BF, tag="ximB", name="ximB"),
        ]
        wh_sb = pers.tile([128, 9 * 128], BF, tag="wh")
        wx_sb = pers.tile([28, 128], BF, tag="wx")
        # elementwise operand placement: every two-input DVE op needs both
        # inputs on the same base partition, so: g at base 0 (own tile),
        # c/prod/tmp at base 32, tanh(c) at base 64 (next to o).
        c_state = pers.tile([64, 8 * 1024], F16, tag="cstate")
        prod = pers.tile([64, 8 * 1024], F16, tag="prod")
        tmp = pers.tile([64, 8 * 1024], F16, tag="tmp")
        tanhc = pers.tile([96, 8 * 1024], F16, tag="tanhc")
        hfin = pers.tile([32, 8 * 1024], F32, tag="hfin")
        zscr = pers.tile([CH, 1024], BF, tag="zscr")

        # ---- init ----
        nc.vector.memset(hstack[0][:, :], 0.0)
        nc.gpsimd.memset(hstack[1][:, :], 0.0)
        nc.vector.memset(c_state[32:64, :], 0.0)
        nc.vector.memset(zscr[:, :], 0.0)
        for i in range(2):
            nc.sync.dma_start(out=xim_sb[i][27:28, :], in_=ones_d[:, :])
        nc.sync.dma_start(out=wx_sb[:, :], in_=wxl_d[:, :])
        for i in range(2):
            nc.sync.dma_start(out=agin[i][2], in_=zscr[:, :])
        for _d in range(9):
            nc.sync.dma_start(out=wh_sb[:, 128 * _d:128 * (_d + 1)],
                              in_=whl_d[_d])
        nc.sync.dma_start(out=xim_sb[0][0:27, :], in_=xim_d[0])

        r_lo = nc.alloc_register(mybir.EngineType.Pool, "r_lo")
        r_hi = nc.alloc_register(mybir.EngineType.Pool, "r_hi")
        nc.reg_load(r_lo, hoff_d[0:1, 0:1])
        nc.reg_load(r_hi, hoff_d[0:1, 1:2])
        rv_lo = nc.snap(r_lo, min_val=0, max_val=11)
        rv_hi = nc.snap(r_hi, min_val=0, max_val=11)

        hsv = [h[:, :].rearrange("p (z y x) -> p z y x", z=SLAB, y=PLW, x=PLW)
               for h in hstack]
        xsv = [x[:, :].rearrange("p (z y x) -> p z y x", z=SLAB, y=PLW, x=PLW)
               for x in xim_sb]

        def emit_plane(t, curv, curx, gates, p):
            """x-conv + 9-delta h-conv matmul accumulation + activations."""
            ps = psum.tile([128, 1024], F32, tag="ps", name="ps")
            for h in range(2):
                nc.tensor.matmul(ps[:, 512 * h:512 * (h + 1)],
                                 lhsT=wx_sb[:, :],
                                 rhs=curx[:, p, 16 * h:16 * h + 16, 0:32],
                                 start=True, stop=(t == 0))
            if t > 0:
                for di, (dy, dx) in enumerate(DELTAS):
                    for h in range(2):
                        nc.tensor.matmul(
                            ps[:, 512 * h:512 * (h + 1)],
                            lhsT=wh_sb[:, 128 * di:128 * (di + 1)],
                            rhs=curv[:, p, 16 * h + dy:16 * h + dy + 16,
                                     dx:dx + 32],
                            start=False, stop=(di == 8))
            s = SPAN[p]
            nc.scalar.activation(gates[0:96, s:s + 1024], ps[0:96, :],
                                 mybir.ActivationFunctionType.Sigmoid)
            nc.scalar.activation(g_t[0:32, s:s + 1024], ps[96:128, :],
                                 mybir.ActivationFunctionType.Tanh)

        def emit_group(t, gates, g_t, nxtv, planes, s0, s1):
            """LSTM elementwise update for gate span [s0:s1] (planes list)."""
            i_sl = gates[0:32, s0:s1]
            f_sl = gates[32:64, s0:s1]
            o_sl = gates[64:96, s0:s1]
            c_sl = c_state[32:64, s0:s1]
            nc.vector.tensor_mul(prod[32:64, s0:s1], i_sl, g_t[0:32, s0:s1])
            nc.vector.tensor_mul(tmp[32:64, s0:s1], f_sl, c_sl)
            nc.vector.tensor_add(c_sl, tmp[32:64, s0:s1], prod[32:64, s0:s1])
            nc.scalar.activation(tanhc[64:96, s0:s1], c_sl,
                                 mybir.ActivationFunctionType.Tanh)
            last = t == nsteps - 1
            for pl, a, b in planes:
                o_ap = o_sl[:, a - s0:b - s0].rearrange(
                    "p (z y x) -> p z y x", z=(b - a) // 1024, y=32, x=32)
                t_ap = tanhc[64:96, a:b].rearrange(
                    "p (z y x) -> p z y x", z=(b - a) // 1024, y=32, x=32)
                if last:
                    nc.vector.tensor_mul(
                        hfin[:, a:b].rearrange("p (z y x) -> p z y x",
                                               z=(b - a) // 1024, y=32, x=32),
                        o_ap, t_ap)
                else:
                    npl = (b - a) // 1024
                    nc.vector.tensor_mul(
                        nxtv[32:64, pl:pl + npl, 1:33, 1:33], o_ap, t_ap)

        T_ = nsteps
        for t in range(T_):
            curv, nxtv = hsv[t % 2], hsv[(t + 1) % 2]
            curx = xsv[t % 2]
            last = t == T_ - 1
            gates = work.tile([96, 8 * 1024], BF, tag="gates", name="gates")
            g_t = work.tile([32, 8 * 1024], BF, tag="g_t", name="g_t")
            if not last:
                nc.sync.dma_start(out=xim_sb[(t + 1) % 2][0:27, :],
                                  in_=xim_d[t + 1])

            # process planes in pair-chunks in PO order; after each chunk,
            # run its elementwise update and issue the dz +-1 replica copies
            # for every slot whose source plane is now available
            ag_i, ag_o = agin[t % 2], agout[t % 2]
            # g0 slot p+1 <- h[p] copy spans, g2 slot p-1 <- h[p], per chunk
            CPY = {
                1: [(0, 2, 4, 1, 3), (2, 0, 2, 1, 3)],
                3: [(0, 4, 6, 3, 5), (2, 2, 4, 3, 5)],
                5: [(0, 6, 8, 5, 7), (2, 4, 6, 5, 7)],
                0: [(0, 1, 2, 0, 1), (2, 6, 7, 7, 8)],
            }
            # chunk processing order: next step starts at plane 3, whose
            # stencil inputs are produced by mid-step chunks of this step
            for p0, p1 in ((3, 4), (5, 6), (1, 2), (0, 7)):
                emit_plane(t, curv, curx, gates, p0)
                emit_plane(t, curv, curx, gates, p1)
                s0 = SPAN[p0]
                planes = ([(p0, s0, s0 + 2048)] if p1 == p0 + 1 else
                          [(p0, s0, s0 + 1024), (p1, s0 + 1024, s0 + 2048)])
                emit_group(t, gates, g_t, nxtv, planes, s0, s0 + 2048)
                if last:
                    continue
                for grp, d0, d1, src0, src1 in (CPY[p0] if copies else []):
                    gp = 0 if grp == 0 else 64
                    nc.scalar.dma_start(out=nxtv[gp:gp + 32, d0:d1, :, :],
                                        in_=nxtv[32:64, src0:src1, :, :])
                if p0 == 0:  # boundary chunk is last: h0 and h7 both ready
                    nc.scalar.dma_start(
                        out=ag_i[0].rearrange("c (y x) -> c y x", y=32, x=32),
                        in_=nxtv[32:64, 0, 1:33, 1:33])
                    nc.scalar.dma_start(
                        out=ag_i[1].rearrange("c (y x) -> c y x", y=32, x=32),
                        in_=nxtv[32:64, 7, 1:33, 1:33])
                    if halo:
                        nc.gpsimd.collective_compute(
                            "AllGather", mybir.AluOpType.bypass,
                            replica_groups=RG,
                            ins=[ag_i[:, :, :]], outs=[ag_o[:, :, :]])

            if not last and halo:
                halo_lo = ag_o[bass.ds(rv_lo, 1)].squeeze(0).rearrange(
                    "c (y x) -> c y x", y=32, x=32)
                halo_hi = ag_o[bass.ds(rv_hi, 1)].squeeze(0).rearrange(
                    "c (y x) -> c y x", y=32, x=32)
                nc.gpsimd.dma_start(out=nxtv[0:32, 0, 1:33, 1:33], in_=halo_lo)
                nc.gpsimd.dma_start(out=nxtv[64:96, 7, 1:33, 1:33], in_=halo_hi)
            else:
                for pl in range(SLAB):
                    s = SPAN[pl]
                    nc.sync.dma_start(
                        out=hout_d[:, pl, :, :],
                        in_=hfin[:, s:s + 1024].rearrange(
                            "c (y x) -> c y x", y=32, x=32))

    nc.finalize()
    _prog_cache[key] = nc
    return nc


def _host_inputs(input_batch, Wx, Wh, b):
    input_batch = np.asarray(input_batch, dtype=np.float32)
    Wx = np.asarray(Wx, dtype=np.float32)
    Wh = np.asarray(Wh, dtype=np.float32)
    b = np.asarray(b, dtype=np.float32)

    xp = np.zeros((2, T, 66, 66, 66), np.float32)
    xp[:, :, 1:65, 1:65, 1:65] = input_batch[:, :, 0]

    whl = np.zeros((9, 128, 128), np.float32)
    for di, (dy, dx) in enumerate(DELTAS):
        for g in range(3):
            whl[di, 32 * g:32 * g + 32, :] = Wh[:, :, g, dy, dx].T
    whl = whl.astype(NPBF)
    wxl = np.zeros((28, 128), np.float32)
    wxl[0:27, :] = Wx[:, 0].reshape(128, 27).T
    wxl[27, :] = b
    wxl = wxl.astype(NPBF)

    in_maps = []
    for c in range(8):
        bidx, k = divmod(c, 4)
        z0 = 8 * k
        xim = np.zeros((T, 27, SLAB, PLW, PLW), np.float32)
        for tz in range(3):
            for ty in range(3):
                for tx in range(3):
                    tap = tz * 9 + ty * 3 + tx
                    xim[:, tap, :, 0:32, 0:32] = xp[
                        bidx, :, 2 * z0 + tz:2 * z0 + tz + 16:2,
                        ty:ty + 64:2, tx:tx + 64:2]
        lo_slot = 3 * k + 2 if k == 0 else 3 * (k - 1) + 1
        hi_slot = 3 * k + 2 if k == 3 else 3 * (k + 1)
        in_maps.append({
            "xim": xim.reshape(T, 27, HS_FREE).astype(NPBF),
            "whl": whl,
            "wxl": wxl,
            "ones": np.ones((1, HS_FREE), NPBF),
            "hoff": np.array([[lo_slot, hi_slot]], np.int32),
        })
    return in_maps


def run_cores(in_maps, nsteps=T, halo=True, copies=True, **kwargs):
    nc = _build_program(nsteps, halo, copies)
    return run_bass_kernel_spmd(nc, in_maps, list(range(8)), **kwargs)


def kernel(input_batch, Wx, Wh, b):
    in_maps = _host_inputs(input_batch, Wx, Wh, b)
    res = run_cores(in_maps)
    out = np.zeros((2, CH, 32, 32, 32), np.float32)
    for c in range(8):
        bidx, k = divmod(c, 4)
        out[bidx, :, 8 * k:8 * k + 8] = res.results[c]["hout"]
    return out
